# revision 4
# baseline (speedup 1.0000x reference)
"""Trainium2 Bass kernel for nn_AIGStateEncoder (2-layer LSTM + linear head).

Data-parallel over batch: B=4096 rows split across 8 NeuronCores (512 each).
Per core the two LSTM layers are fused into one recurrence (combined step s
runs layer0 at t=s and layer1 at t=s-1) with a single state tile
U = [h0 (0:64); h1 (64:128)] kept transposed (hidden on partitions, batch on
the free dim); two independent batch groups of 256 pipeline the serial chain.

Per gate ONE M=128 matmul over U with the block weight matrix
[[W_hh0, W_ih1], [0, W_hh1]] produces both layers' pre-activations, and one
K=2 matmul of [x_t; 1] against [W_ih0 | 0; b0 | b1] folds the scalar input
AND the biases into PSUM.  Gates f,i share a PSUM bank ([f | i], 512 f32) so
ONE sigmoid ACTIVATE covers both layers of both gates; g,o share a second
bank evaluated by ONE tanh ACTIVATE using the identity
sigma(z) = 0.5*(1+tanh(z/2)): the o-gate weights are pre-halved and the h
written to U is 2h, compensated by halving every U-consuming weight.

Per group-step: Scalar: sigmoid[512], tanh[512], tanh(c)[256]; DVE:
t12 = sFI*[c|g] (512-wide TT) and h = (to+1)*tanh(c) (STT); GpSimd:
c = t1 + t2.  Everything computes in bf16 (fp32 PSUM).
"""
import sys

if '/opt/trn_rl_repo' not in sys.path:
    sys.path.insert(0, '/opt/trn_rl_repo')

import numpy as np
import ml_dtypes

B, T, H = 4096, 256, 64
N_CORES = 8
B_LOC = B // N_CORES  # 512
NG = 2
NB = B_LOC // NG      # 256

# weight col-block order: f, i (bank FI), g, o (bank GO)
GATES = ("f", "i", "g", "o")
TORCH_OFF = {"i": 0, "f": H, "g": 2 * H, "o": 3 * H}


def _split_excess_waits(nc, limit=1):
    """The walrus build in this container accepts at most one sync wait per
    instruction; hoist excess waits onto same-engine NoOps."""
    import concourse.mybir as mybir
    ctr = 0
    for f in nc.m.functions:
        for bb in f.blocks:
            il = bb.instructions
            i = 0
            while i < len(il):
                ins = il[i]
                si = ins.sync_info
                if si is not None and si.on_wait and len(si.on_wait) > limit:
                    waits = list(si.on_wait)
                    excess, keep = waits[:-limit], waits[-limit:]
                    while excess:
                        chunk, excess = excess[:limit], excess[limit:]
                        nop = mybir.InstNoOp(name=f"waitsplit_{ctr}", ins=[], outs=[])
                        ctr += 1
                        nop.engine = ins.engine
                        nop.sync_info = mybir.SyncInfo(on_wait=chunk, on_update=[])
                        il.insert(i, nop)
                        i += 1
                    ins.sync_info = mybir.SyncInfo(on_wait=keep,
                                                   on_update=list(si.on_update))
                i += 1


def _build_program():
    import concourse.bass as bass
    import concourse.mybir as mybir
    from concourse.tile import TileContext

    BF16 = mybir.dt.bfloat16
    F32 = mybir.dt.float32
    AF = mybir.ActivationFunctionType
    OP = mybir.AluOpType

    nc = bass.Bass()
    xT = nc.declare_dram_parameter("xT", [T, B_LOC], BF16, isOutput=False)
    wmain = nc.declare_dram_parameter("wmain", [2 * H, 8 * H], BF16, isOutput=False)
    wx = nc.declare_dram_parameter("wx", [2, 8 * H], BF16, isOutput=False)
    wx0 = nc.declare_dram_parameter("wx0", [2, 8 * H], BF16, isOutput=False)
    wxT = nc.declare_dram_parameter("wxT", [2, 8 * H], BF16, isOutput=False)
    wlin = nc.declare_dram_parameter("wlin", [H, H], BF16, isOutput=False)
    blin = nc.declare_dram_parameter("blin", [2 * H, H], F32, isOutput=False)
    out = nc.declare_dram_parameter("out", [B_LOC, H], F32, isOutput=True)

    with TileContext(nc) as tc:
        with (
            tc.tile_pool(name="const", bufs=1) as cpool,
            tc.tile_pool(name="state", bufs=1) as spool,
            tc.tile_pool(name="work", bufs=3) as wpool,
            tc.tile_pool(name="psum", bufs=1, space="PSUM") as ppool,
        ):
            wm_s = cpool.tile([2 * H, 8 * H], BF16, tag="wm", name="wm")
            nc.sync.dma_start(wm_s[:], wmain[:])
            wx_s = cpool.tile([2, 8 * H], BF16, tag="wxm", name="wxm")
            nc.sync.dma_start(wx_s[:], wx[:])
            wx0_s = cpool.tile([2, 8 * H], BF16, tag="wx0", name="wx0s")
            nc.sync.dma_start(wx0_s[:], wx0[:])
            wxT_s = cpool.tile([2, 8 * H], BF16, tag="wxT", name="wxTs")
            nc.sync.dma_start(wxT_s[:], wxT[:])
            wlin_s = cpool.tile([H, H], BF16, tag="wlin", name="wlin_s")
            nc.sync.dma_start(wlin_s[:], wlin[:])
            blin_s = cpool.tile([2 * H, H], F32, tag="blin", name="blin_s")
            nc.sync.dma_start(blin_s[:], blin[:])

            U = []      # [128, NB] state [h0*2; h1*2], ping/pong
            xr = []     # [2, NB] moving operand [x_t; 1], ping/pong
            cg = []     # [128, 3*NB]: [c | tanh(g) | tanh(o'/...)]
            sFI = []    # [128, 2*NB]: [sigma(f) | sigma(i)]
            t12 = []    # [128, 2*NB]: [f*c | i*g]
            tC = []     # [128, NB]: tanh(c)
            for g in range(NG):
                U.append([spool.tile([2 * H, NB], BF16, tag=f"U{g}_{p}",
                                     name=f"U{g}_{p}") for p in range(2)])
                xr.append([spool.tile([2, NB], BF16, tag=f"xr{g}_{p}",
                                      name=f"xr{g}_{p}") for p in range(2)])
                cg.append(spool.tile([2 * H, 3 * NB], BF16, tag=f"cg{g}",
                                     name=f"cg{g}"))
                sFI.append(spool.tile([2 * H, 2 * NB], BF16, tag=f"sFI{g}",
                                      name=f"sFI{g}"))
                t12.append(spool.tile([2 * H, 2 * NB], BF16, tag=f"t12{g}",
                                      name=f"t12{g}"))
                tC.append(spool.tile([2 * H, NB], BF16, tag=f"tC{g}",
                                     name=f"tC{g}"))
                for p in range(2):
                    nc.gpsimd.memset(U[g][p][:], 0.0)
                    nc.gpsimd.memset(xr[g][p][:], 1.0)
                nc.gpsimd.memset(cg[g][:], 0.0)
                nc.sync.dma_start(xr[g][0][0:1, :], xT[0:1, g * NB:(g + 1) * NB])

            bankFI = [ppool.tile([2 * H, 2 * NB], F32, tag=f"bankFI{g}",
                                 name=f"bankFI{g}") for g in range(NG)]
            bankGO = [ppool.tile([2 * H, 2 * NB], F32, tag=f"bankGO{g}",
                                 name=f"bankGO{g}") for g in range(NG)]

            def step(s):
                cur, nxt = s % 2, (s + 1) % 2
                wxv = wx0_s if s == 0 else (wxT_s if s == T else wx_s)
                # --- PE: bankFI = [f | i], bankGO = [g | o'] ---
                for g in range(NG):
                    for bank, pair in ((bankFI, ("f", "i")), (bankGO, ("g", "o"))):
                        for bi, gate in enumerate(pair):
                            gc = GATES.index(gate) * 2 * H
                            dst = bank[g][:, bi * NB:(bi + 1) * NB]
                            nc.tensor.matmul(dst, wm_s[:, gc:gc + 2 * H],
                                             U[g][cur][:], start=True, stop=False)
                            nc.tensor.matmul(dst, wxv[:, gc:gc + 2 * H],
                                             xr[g][cur][:], start=False, stop=True)
                # --- Scalar: sigmoid over [f | i]; tanh over [g | o'] ---
                for g in range(NG):
                    nc.scalar.activation(sFI[g][:], bankFI[g][:], AF.Sigmoid)
                for g in range(NG):
                    nc.scalar.activation(cg[g][:, NB:3 * NB], bankGO[g][:], AF.Tanh)
                # --- DVE: t12 = sFI * [c | tanh(g)] ---
                for g in range(NG):
                    nc.vector.tensor_tensor(t12[g][:], sFI[g][:],
                                            cg[g][:, 0:2 * NB], op=OP.mult)
                # --- Pool: c' = t1 + t2 ---
                for g in range(NG):
                    nc.gpsimd.tensor_tensor(cg[g][:, 0:NB], t12[g][:, 0:NB],
                                            t12[g][:, NB:2 * NB], op=OP.add)
                # --- Scalar: tC = tanh(c') ---
                for g in range(NG):
                    nc.scalar.activation(tC[g][:], cg[g][:, 0:NB], AF.Tanh)
                # --- DVE: U[nxt] = (tanh(o') + 1) * tC  (= 2h) ---
                for g in range(NG):
                    nc.vector.scalar_tensor_tensor(U[g][nxt][:],
                                                   cg[g][:, 2 * NB:3 * NB], 1.0,
                                                   tC[g][:],
                                                   op0=OP.add, op1=OP.mult)
                # --- DMA next x row ---
                if s + 1 < T:
                    for g in range(NG):
                        nc.sync.dma_start(xr[g][nxt][0:1, :],
                                          xT[s + 1:s + 2, g * NB:(g + 1) * NB])

            for s in range(T + 1):
                step(s)

            # final linear: out[b, :] = (2*h1).T @ (wlin/2) + blin
            fin = (T + 1) % 2
            h1f = [spool.tile([H, NB], BF16, tag=f"h1f{g}", name=f"h1f{g}")
                   for g in range(NG)]
            for g in range(NG):
                nc.sync.dma_start(h1f[g][:], U[g][fin][H:2 * H, :])
            for g in range(NG):
                for blk in range(NB // 128):
                    psl = ppool.tile([128, H], F32, tag="ps_f0", name="psl")
                    nc.tensor.matmul(psl[:],
                                     h1f[g][:, blk * 128:(blk + 1) * 128],
                                     wlin_s[:], start=True, stop=True)
                    ob = wpool.tile([128, H], F32, tag="ob", name="ob")
                    nc.vector.scalar_tensor_tensor(ob[:], psl[:], 1.0,
                                                   blin_s[0:128, :],
                                                   op0=OP.mult, op1=OP.add)
                    row0 = g * NB + blk * 128
                    nc.sync.dma_start(out[row0:row0 + 128, :], ob[:])

    _split_excess_waits(nc, limit=1)
    return nc


def _prep_inputs(inputs):
    bf = ml_dtypes.bfloat16
    f32 = np.float32
    recipe = np.ascontiguousarray(np.asarray(inputs["recipe"], f32).reshape(B, T))
    W_ih0 = np.asarray(inputs["W_ih0"], f32)   # [4H, 1]
    W_hh0 = np.asarray(inputs["W_hh0"], f32)   # [4H, H]
    W_ih1 = np.asarray(inputs["W_ih1"], f32)   # [4H, H]
    W_hh1 = np.asarray(inputs["W_hh1"], f32)   # [4H, H]
    b0 = np.asarray(inputs["b_ih0"], f32) + np.asarray(inputs["b_hh0"], f32)
    b1 = np.asarray(inputs["b_ih1"], f32) + np.asarray(inputs["b_hh1"], f32)

    # U holds 2h -> all weights contracting h are halved; o-gate rows are
    # additionally halved (tanh(z/2) identity).
    wmain = np.zeros((2 * H, 8 * H), f32)
    wxr = np.zeros((2, 8 * H), f32)
    for gi, gate in enumerate(GATES):
        off = TORCH_OFF[gate]
        c0 = gi * 2 * H
        gsc = 0.5 if gate == "o" else 1.0   # pre-activation scale (o: z/2)
        usc = 0.5                           # U holds 2h
        wmain[0:H, c0:c0 + H] = W_hh0[off:off + H, :].T * gsc * usc
        wmain[0:H, c0 + H:c0 + 2 * H] = W_ih1[off:off + H, :].T * gsc * usc
        wmain[H:2 * H, c0 + H:c0 + 2 * H] = W_hh1[off:off + H, :].T * gsc * usc
        wxr[0, c0:c0 + H] = W_ih0[off:off + H, 0] * gsc
        wxr[1, c0:c0 + H] = b0[off:off + H] * gsc
        wxr[1, c0 + H:c0 + 2 * H] = b1[off:off + H] * gsc
    wx0 = wxr.copy()
    wxT = wxr.copy()
    for gi in range(4):
        c0 = gi * 2 * H
        wx0[1, c0 + H:c0 + 2 * H] = 0.0        # s=0: no layer1 bias
        wxT[0, c0:c0 + H] = 0.0                # s=T: no layer0 input
        wxT[1, c0:c0 + H] = 0.0                # s=T: no layer0 bias

    wlin = np.ascontiguousarray(np.asarray(inputs["W_lin"], f32).T * 0.5).astype(bf)
    blin = np.tile(np.asarray(inputs["b_lin"], f32), (2 * H, 1))

    common = {"wmain": wmain.astype(bf), "wx": wxr.astype(bf),
              "wx0": wx0.astype(bf), "wxT": wxT.astype(bf),
              "wlin": wlin, "blin": blin}
    in_maps = []
    for i in range(N_CORES):
        shard = recipe[i * B_LOC:(i + 1) * B_LOC]
        xTs = np.ascontiguousarray(shard.T).astype(bf)
        in_maps.append({"xT": xTs, **common})
    return in_maps


_PROGRAM = []


def _run(inputs, trace=False):
    from concourse.bass_utils import run_bass_kernel_spmd
    if not _PROGRAM:
        _PROGRAM.append(_build_program())
    nc = _PROGRAM[0]
    in_maps = _prep_inputs(inputs)
    last_err = None
    for attempt in range(3):
        try:
            res = run_bass_kernel_spmd(nc, in_maps,
                                       core_ids=list(range(N_CORES)), trace=trace)
            outs = [np.asarray(res.results[i]["out"]) for i in range(N_CORES)]
            return np.concatenate(outs, axis=0), res
        except Exception as e:  # transient first-exec device faults: retry
            last_err = e
    raise last_err


def kernel(**inputs):
    full, _ = _run(inputs, trace=False)
    return full.astype(np.float32)


# revision 7
# speedup vs baseline: 1.0824x; 1.0824x over previous
"""Trainium2 Bass kernel for nn_AIGStateEncoder (2-layer LSTM + linear head).

Data-parallel over batch: B=4096 rows split across 8 NeuronCores (512 each).
Per core the two LSTM layers are fused into one recurrence (combined step s
runs layer0 at t=s and layer1 at t=s-1) with a single state tile
U = [h0 (0:64); h1 (64:128)] kept transposed (hidden on partitions, batch on
the free dim); two independent batch groups of 256 pipeline the serial chain.

Per gate ONE M=128 matmul over U with the block weight matrix
[[W_hh0, W_ih1], [0, W_hh1]] produces both layers' pre-activations, and one
K=2 matmul of [x_t; 1] against [W_ih0 | 0; b0 | b1] folds the scalar input
AND the biases into PSUM.  Gates f,i share a PSUM bank ([f | i], 512 f32) so
ONE sigmoid ACTIVATE covers both layers of both gates; g,o share a second
bank evaluated by ONE tanh ACTIVATE using the identity
sigma(z) = 0.5*(1+tanh(z/2)): the o-gate weights are pre-halved and the h
written to U is 2h, compensated by halving every U-consuming weight.

Per group-step: Scalar: sigmoid[512], tanh[512], tanh(c)[256]; DVE:
t12 = sFI*[c|g] (512-wide TT) and h = (to+1)*tanh(c) (STT); GpSimd:
c = t1 + t2.  Everything computes in bf16 (fp32 PSUM).
"""
import sys

if '/opt/trn_rl_repo' not in sys.path:
    sys.path.insert(0, '/opt/trn_rl_repo')

import numpy as np
import ml_dtypes

B, T, H = 4096, 256, 64
N_CORES = 8
B_LOC = B // N_CORES  # 512
NG = 2
NB = B_LOC // NG      # 256

# weight col-block order: f, i (bank FI), g, o (bank GO)
GATES = ("f", "i", "g", "o")
TORCH_OFF = {"i": 0, "f": H, "g": 2 * H, "o": 3 * H}


def _split_excess_waits(nc, limit=1):
    """The walrus build in this container accepts at most one sync wait per
    instruction; hoist excess waits onto same-engine NoOps."""
    import concourse.mybir as mybir
    ctr = 0
    for f in nc.m.functions:
        for bb in f.blocks:
            il = bb.instructions
            i = 0
            while i < len(il):
                ins = il[i]
                si = ins.sync_info
                if si is not None and si.on_wait and len(si.on_wait) > limit:
                    waits = list(si.on_wait)
                    excess, keep = waits[:-limit], waits[-limit:]
                    while excess:
                        chunk, excess = excess[:limit], excess[limit:]
                        nop = mybir.InstNoOp(name=f"waitsplit_{ctr}", ins=[], outs=[])
                        ctr += 1
                        nop.engine = ins.engine
                        nop.sync_info = mybir.SyncInfo(on_wait=chunk, on_update=[])
                        il.insert(i, nop)
                        i += 1
                    ins.sync_info = mybir.SyncInfo(on_wait=keep,
                                                   on_update=list(si.on_update))
                i += 1


def _build_program():
    import concourse.bass as bass
    import concourse.mybir as mybir
    from concourse.tile import TileContext

    BF16 = mybir.dt.bfloat16
    F32 = mybir.dt.float32
    AF = mybir.ActivationFunctionType
    OP = mybir.AluOpType

    nc = bass.Bass()
    xT = nc.declare_dram_parameter("xT", [T, B_LOC], BF16, isOutput=False)
    wmain = nc.declare_dram_parameter("wmain", [2 * H, 8 * H], BF16, isOutput=False)
    wx = nc.declare_dram_parameter("wx", [2, 8 * H], BF16, isOutput=False)
    wx0 = nc.declare_dram_parameter("wx0", [2, 8 * H], BF16, isOutput=False)
    wxT = nc.declare_dram_parameter("wxT", [2, 8 * H], BF16, isOutput=False)
    wlin = nc.declare_dram_parameter("wlin", [H, H], BF16, isOutput=False)
    blin = nc.declare_dram_parameter("blin", [2 * H, H], F32, isOutput=False)
    out = nc.declare_dram_parameter("out", [B_LOC, H], F32, isOutput=True)

    with TileContext(nc) as tc:
        with (
            tc.tile_pool(name="const", bufs=1) as cpool,
            tc.tile_pool(name="state", bufs=1) as spool,
            tc.tile_pool(name="work", bufs=3) as wpool,
            tc.tile_pool(name="psum", bufs=1, space="PSUM") as ppool,
        ):
            wm_s = cpool.tile([2 * H, 8 * H], BF16, tag="wm", name="wm")
            nc.sync.dma_start(wm_s[:], wmain[:])
            wx_s = cpool.tile([2, 8 * H], BF16, tag="wxm", name="wxm")
            nc.sync.dma_start(wx_s[:], wx[:])
            wx0_s = cpool.tile([2, 8 * H], BF16, tag="wx0", name="wx0s")
            nc.sync.dma_start(wx0_s[:], wx0[:])
            wxT_s = cpool.tile([2, 8 * H], BF16, tag="wxT", name="wxTs")
            nc.sync.dma_start(wxT_s[:], wxT[:])
            wlin_s = cpool.tile([H, H], BF16, tag="wlin", name="wlin_s")
            nc.sync.dma_start(wlin_s[:], wlin[:])
            blin_s = cpool.tile([2 * H, H], F32, tag="blin", name="blin_s")
            nc.sync.dma_start(blin_s[:], blin[:])

            U = []      # [128, NB] state [h0*2; h1*2], ping/pong
            xr = []     # [2, NB] moving operand [x_t; 1], ping/pong
            cg = []     # [128, 3*NB]: [c | tanh(g) | tanh(o'/...)]
            sFI = []    # [128, 2*NB]: [sigma(f) | sigma(i)]
            t12 = []    # [128, 2*NB]: [f*c | i*g]
            tC = []     # [128, NB]: tanh(c)
            for g in range(NG):
                U.append([spool.tile([2 * H, NB], BF16, tag=f"U{g}_{p}",
                                     name=f"U{g}_{p}") for p in range(2)])
                xr.append([spool.tile([2, NB], BF16, tag=f"xr{g}_{p}",
                                      name=f"xr{g}_{p}") for p in range(2)])
                cg.append(spool.tile([2 * H, 3 * NB], BF16, tag=f"cg{g}",
                                     name=f"cg{g}"))
                sFI.append(spool.tile([2 * H, 2 * NB], BF16, tag=f"sFI{g}",
                                      name=f"sFI{g}"))
                t12.append(spool.tile([2 * H, 2 * NB], BF16, tag=f"t12{g}",
                                      name=f"t12{g}"))
                tC.append(spool.tile([2 * H, NB], BF16, tag=f"tC{g}",
                                     name=f"tC{g}"))
                for p in range(2):
                    nc.gpsimd.memset(U[g][p][:], 0.0)
                    nc.gpsimd.memset(xr[g][p][:], 1.0)
                nc.gpsimd.memset(cg[g][:], 0.0)
                nc.sync.dma_start(xr[g][0][0:1, :], xT[0:1, g * NB:(g + 1) * NB])

            bankFI = [ppool.tile([2 * H, 2 * NB], F32, tag=f"bankFI{g}",
                                 name=f"bankFI{g}") for g in range(NG)]
            bankGO = [ppool.tile([2 * H, 2 * NB], F32, tag=f"bankGO{g}",
                                 name=f"bankGO{g}") for g in range(NG)]

            def step(s):
                cur, nxt = s % 2, (s + 1) % 2
                wxv = wx0_s if s == 0 else (wxT_s if s == T else wx_s)
                # --- PE: bankFI = [f | i], bankGO = [g | o'] ---
                # Bank-alternating order: each main->x accumulate pair stays
                # sequential within its bank (PSUM group semantics) but is
                # separated by a matmul on the other bank so the PE pipe
                # never stalls on a same-region dependent pair.
                for g in range(NG):
                    for bi in range(2):
                        gate_fi, gate_go = ("f", "g") if bi == 0 else ("i", "o")
                        for mi in range(2):  # 0: main over U, 1: [x;1] over xr
                            for bank, gate in ((bankFI, gate_fi),
                                               (bankGO, gate_go)):
                                gc = GATES.index(gate) * 2 * H
                                dst = bank[g][:, bi * NB:(bi + 1) * NB]
                                if mi == 0:
                                    nc.tensor.matmul(dst, wm_s[:, gc:gc + 2 * H],
                                                     U[g][cur][:],
                                                     start=True, stop=False)
                                else:
                                    nc.tensor.matmul(dst, wxv[:, gc:gc + 2 * H],
                                                     xr[g][cur][:],
                                                     start=False, stop=True)
                # --- Scalar: sigmoid over [f | i]; tanh over [g | o'] ---
                for g in range(NG):
                    nc.scalar.activation(sFI[g][:], bankFI[g][:], AF.Sigmoid)
                for g in range(NG):
                    nc.scalar.activation(cg[g][:, NB:3 * NB], bankGO[g][:], AF.Tanh)
                # --- DVE: t12 = sFI * [c | tanh(g)] ---
                for g in range(NG):
                    nc.vector.tensor_tensor(t12[g][:], sFI[g][:],
                                            cg[g][:, 0:2 * NB], op=OP.mult)
                # --- DVE: c' = t1 + t2 ---
                for g in range(NG):
                    nc.vector.tensor_tensor(cg[g][:, 0:NB], t12[g][:, 0:NB],
                                            t12[g][:, NB:2 * NB], op=OP.add)
                # --- Scalar: tC = tanh(c') ---
                for g in range(NG):
                    nc.scalar.activation(tC[g][:], cg[g][:, 0:NB], AF.Tanh)
                # --- DVE: U[nxt] = (tanh(o') + 1) * tC  (= 2h) ---
                for g in range(NG):
                    nc.vector.scalar_tensor_tensor(U[g][nxt][:],
                                                   cg[g][:, 2 * NB:3 * NB], 1.0,
                                                   tC[g][:],
                                                   op0=OP.add, op1=OP.mult)
                # --- DMA next x row ---
                if s + 1 < T:
                    for g in range(NG):
                        nc.sync.dma_start(xr[g][nxt][0:1, :],
                                          xT[s + 1:s + 2, g * NB:(g + 1) * NB])

            for s in range(T + 1):
                step(s)

            # final linear: out[b, :] = (2*h1).T @ (wlin/2) + blin
            fin = (T + 1) % 2
            h1f = [spool.tile([H, NB], BF16, tag=f"h1f{g}", name=f"h1f{g}")
                   for g in range(NG)]
            for g in range(NG):
                nc.sync.dma_start(h1f[g][:], U[g][fin][H:2 * H, :])
            for g in range(NG):
                for blk in range(NB // 128):
                    psl = ppool.tile([128, H], F32, tag="ps_f0", name="psl")
                    nc.tensor.matmul(psl[:],
                                     h1f[g][:, blk * 128:(blk + 1) * 128],
                                     wlin_s[:], start=True, stop=True)
                    ob = wpool.tile([128, H], F32, tag="ob", name="ob")
                    nc.vector.scalar_tensor_tensor(ob[:], psl[:], 1.0,
                                                   blin_s[0:128, :],
                                                   op0=OP.mult, op1=OP.add)
                    row0 = g * NB + blk * 128
                    nc.sync.dma_start(out[row0:row0 + 128, :], ob[:])

    _split_excess_waits(nc, limit=1)
    return nc


def _prep_inputs(inputs):
    bf = ml_dtypes.bfloat16
    f32 = np.float32
    recipe = np.ascontiguousarray(np.asarray(inputs["recipe"], f32).reshape(B, T))
    W_ih0 = np.asarray(inputs["W_ih0"], f32)   # [4H, 1]
    W_hh0 = np.asarray(inputs["W_hh0"], f32)   # [4H, H]
    W_ih1 = np.asarray(inputs["W_ih1"], f32)   # [4H, H]
    W_hh1 = np.asarray(inputs["W_hh1"], f32)   # [4H, H]
    b0 = np.asarray(inputs["b_ih0"], f32) + np.asarray(inputs["b_hh0"], f32)
    b1 = np.asarray(inputs["b_ih1"], f32) + np.asarray(inputs["b_hh1"], f32)

    # U holds 2h -> all weights contracting h are halved; o-gate rows are
    # additionally halved (tanh(z/2) identity).
    wmain = np.zeros((2 * H, 8 * H), f32)
    wxr = np.zeros((2, 8 * H), f32)
    for gi, gate in enumerate(GATES):
        off = TORCH_OFF[gate]
        c0 = gi * 2 * H
        gsc = 0.5 if gate == "o" else 1.0   # pre-activation scale (o: z/2)
        usc = 0.5                           # U holds 2h
        wmain[0:H, c0:c0 + H] = W_hh0[off:off + H, :].T * gsc * usc
        wmain[0:H, c0 + H:c0 + 2 * H] = W_ih1[off:off + H, :].T * gsc * usc
        wmain[H:2 * H, c0 + H:c0 + 2 * H] = W_hh1[off:off + H, :].T * gsc * usc
        wxr[0, c0:c0 + H] = W_ih0[off:off + H, 0] * gsc
        wxr[1, c0:c0 + H] = b0[off:off + H] * gsc
        wxr[1, c0 + H:c0 + 2 * H] = b1[off:off + H] * gsc
    wx0 = wxr.copy()
    wxT = wxr.copy()
    for gi in range(4):
        c0 = gi * 2 * H
        wx0[1, c0 + H:c0 + 2 * H] = 0.0        # s=0: no layer1 bias
        wxT[0, c0:c0 + H] = 0.0                # s=T: no layer0 input
        wxT[1, c0:c0 + H] = 0.0                # s=T: no layer0 bias

    wlin = np.ascontiguousarray(np.asarray(inputs["W_lin"], f32).T * 0.5).astype(bf)
    blin = np.tile(np.asarray(inputs["b_lin"], f32), (2 * H, 1))

    common = {"wmain": wmain.astype(bf), "wx": wxr.astype(bf),
              "wx0": wx0.astype(bf), "wxT": wxT.astype(bf),
              "wlin": wlin, "blin": blin}
    in_maps = []
    for i in range(N_CORES):
        shard = recipe[i * B_LOC:(i + 1) * B_LOC]
        xTs = np.ascontiguousarray(shard.T).astype(bf)
        in_maps.append({"xT": xTs, **common})
    return in_maps


_PROGRAM = []


def _run(inputs, trace=False):
    from concourse.bass_utils import run_bass_kernel_spmd
    if not _PROGRAM:
        _PROGRAM.append(_build_program())
    nc = _PROGRAM[0]
    in_maps = _prep_inputs(inputs)
    last_err = None
    for attempt in range(3):
        try:
            res = run_bass_kernel_spmd(nc, in_maps,
                                       core_ids=list(range(N_CORES)), trace=trace)
            outs = [np.asarray(res.results[i]["out"]) for i in range(N_CORES)]
            return np.concatenate(outs, axis=0), res
        except Exception as e:  # transient first-exec device faults: retry
            last_err = e
    raise last_err


def kernel(**inputs):
    full, _ = _run(inputs, trace=False)
    return full.astype(np.float32)


# revision 11
# speedup vs baseline: 1.4292x; 1.3204x over previous
"""Trainium2 Bass kernel for nn_AIGStateEncoder (2-layer LSTM + linear head).

Data-parallel over batch: B=4096 rows split across 8 NeuronCores (512 each).
Per core the two LSTM layers are fused into one recurrence ("combined step"
s runs layer0 at t=s and layer1 at t=s-1), with the state kept transposed
(hidden units on SBUF partitions, batch on the free dimension) and the two
layers stacked on the 128 partitions: [layer0 (0:64); layer1 (64:128)].

Per combined step and batch-group (2 independent groups of 256 batch rows
hide each other's serial-dependency chain):
  - 8 matmuls (4 gates x 2 layers; K=65 for layer0 ([x_t; h0]), K=128 for
    layer1 ([h0; h1]); N=256) into per-gate PSUM banks,
  - 4 sigmoid/tanh ScalarE activations with per-partition bias (both layers
    in one [128, 256] instruction each),
  - cell/hidden updates on VectorE in bf16 (2x mode),
  - tanh(c) on ScalarE.
The per-step x row is DMA'd from DRAM into partition 64 of the layer0
moving operand; h0 is copied into it by VectorE.

Everything computes in bf16 (fp32 PSUM accumulation); measured end-to-end
relative error vs the fp32 reference is ~3e-3.
"""
import sys

if '/opt/trn_rl_repo' not in sys.path:
    sys.path.insert(0, '/opt/trn_rl_repo')

import numpy as np
import ml_dtypes

B, T, H = 4096, 256, 64
N_CORES = 8
B_LOC = B // N_CORES  # 512

GATE_ORDER = ("f", "g", "i", "o")   # emission order (c-critical gates first)
GCOL = {"i": 0, "f": 1, "g": 2, "o": 3}  # PyTorch gate order i,f,g,o


def _split_excess_waits(nc, limit=1):
    """The walrus build in this container accepts at most one sync wait per
    instruction.  Hoist excess waits onto NoOps inserted just before the
    instruction on the same engine (same-engine program order preserves the
    synchronization semantics)."""
    import concourse.mybir as mybir
    ctr = 0
    for f in nc.m.functions:
        for bb in f.blocks:
            il = bb.instructions
            i = 0
            while i < len(il):
                ins = il[i]
                si = ins.sync_info
                if si is not None and si.on_wait and len(si.on_wait) > limit:
                    waits = list(si.on_wait)
                    excess, keep = waits[:-limit], waits[-limit:]
                    while excess:
                        chunk, excess = excess[:limit], excess[limit:]
                        nop = mybir.InstNoOp(name=f"waitsplit_{ctr}", ins=[], outs=[])
                        ctr += 1
                        nop.engine = ins.engine
                        nop.sync_info = mybir.SyncInfo(on_wait=chunk, on_update=[])
                        il.insert(i, nop)
                        i += 1
                    ins.sync_info = mybir.SyncInfo(on_wait=keep,
                                                   on_update=list(si.on_update))
                i += 1


def _build_program(n_groups=2):
    import concourse.bass as bass
    import concourse.mybir as mybir
    from concourse.tile import TileContext

    BF16 = mybir.dt.bfloat16
    F32 = mybir.dt.float32
    AF = mybir.ActivationFunctionType
    OP = mybir.AluOpType
    GFUNC = {"i": AF.Sigmoid, "f": AF.Sigmoid, "g": AF.Tanh, "o": AF.Sigmoid}

    NG = n_groups
    NB = B_LOC // NG

    nc = bass.Bass()
    xT = nc.declare_dram_parameter("xT", [T, B_LOC], BF16, isOutput=False)
    w0 = nc.declare_dram_parameter("w0", [1 + H, 4 * H], BF16, isOutput=False)
    w1 = nc.declare_dram_parameter("w1", [2 * H, 4 * H], BF16, isOutput=False)
    bias = nc.declare_dram_parameter("bias", [2 * H, 4], F32, isOutput=False)
    wlin = nc.declare_dram_parameter("wlin", [H, H], BF16, isOutput=False)
    blin = nc.declare_dram_parameter("blin", [2 * H, H], F32, isOutput=False)
    out = nc.declare_dram_parameter("out", [B_LOC, H], F32, isOutput=True)

    with TileContext(nc) as tc:
        with (
            tc.tile_pool(name="const", bufs=1) as cpool,
            tc.tile_pool(name="state", bufs=1) as spool,
            tc.tile_pool(name="work", bufs=3) as wpool,
            tc.tile_pool(name="psum", bufs=1, space="PSUM") as ppool,
        ):
            w0_s = cpool.tile([1 + H, 4 * H], BF16, tag="w0", name="w0")
            nc.sync.dma_start(w0_s[:], w0[:])
            w1_s = cpool.tile([2 * H, 4 * H], BF16, tag="w1", name="w1")
            nc.sync.dma_start(w1_s[:], w1[:])
            bias_s = cpool.tile([2 * H, 4], F32, tag="bias", name="bias")
            nc.sync.dma_start(bias_s[:], bias[:])
            wlin_s = cpool.tile([H, H], BF16, tag="wlin", name="wlin")
            nc.sync.dma_start(wlin_s[:], wlin[:])
            blin_s = cpool.tile([2 * H, H], F32, tag="blin", name="blin")
            nc.sync.dma_start(blin_s[:], blin[:])

            rhsA = []  # [65, NB] : [h0 (0:64); x_t (64)]
            rhsB = []  # [128, NB]: [h0; h1]
            c_st = []  # [128, NB]: [c0; c1]
            h1f = []   # [64, NB] : final h1
            for g in range(NG):
                rhsA.append([spool.tile([1 + H, NB], BF16, tag=f"rhsA{g}_{p}",
                                        name=f"rhsA{g}_{p}") for p in range(2)])
                rhsB.append([spool.tile([2 * H, NB], BF16, tag=f"rhsB{g}_{p}",
                                        name=f"rhsB{g}_{p}") for p in range(2)])
                c_st.append(spool.tile([2 * H, NB], BF16, tag=f"c{g}", name=f"cst{g}"))
                h1f.append(spool.tile([H, NB], BF16, tag=f"h1f{g}", name=f"h1f{g}"))
                for p in range(2):
                    nc.gpsimd.memset(rhsA[g][p][:], 0.0)
                    nc.gpsimd.memset(rhsB[g][p][:], 0.0)
                nc.gpsimd.memset(c_st[g][:], 0.0)
                nc.sync.dma_start(rhsA[g][0][H:H + 1, :],
                                  xT[0:1, g * NB:(g + 1) * NB])

            def step(s):
                cur, nxt = s % 2, (s + 1) % 2
                l0 = s < T
                l1 = s >= 1
                lo = 0 if l0 else H
                hi = 2 * H if l1 else H
                for g in range(NG):
                    ps = {}
                    for gt in GATE_ORDER:
                        ps[gt] = ppool.tile([2 * H, NB], F32, tag=f"ps_{gt}{g}",
                                            name=f"ps_{gt}{g}")
                        col = GCOL[gt] * H
                        if l0:
                            nc.tensor.matmul(ps[gt][0:H, :], w0_s[:, col:col + H],
                                             rhsA[g][cur][:], start=True, stop=True)
                        if l1:
                            nc.tensor.matmul(ps[gt][H:2 * H, :], w1_s[:, col:col + H],
                                             rhsB[g][cur][:], start=True, stop=True)
                    # f,o: in-place PSUM activations (ScalarE is closer to
                    # PSUM; 172-cycle access vs 222 for SBUF) -- their DVE
                    # consumers pair them with an SBUF operand.  i,g keep
                    # SBUF outputs (t2 = i*g may read only one PSUM operand).
                    act = {}
                    for gt in GATE_ORDER:
                        if gt in ("f", "o"):
                            nc.scalar.activation(ps[gt][lo:hi, :], ps[gt][lo:hi, :],
                                                 GFUNC[gt],
                                                 bias=bias_s[lo:hi, GCOL[gt]:GCOL[gt] + 1])
                            act[gt] = ps[gt]
                        else:
                            a = wpool.tile([2 * H, NB], BF16, tag=f"a_{gt}{g}",
                                           name=f"a_{gt}{g}")
                            nc.scalar.activation(a[lo:hi, :], ps[gt][lo:hi, :],
                                                 GFUNC[gt],
                                                 bias=bias_s[lo:hi, GCOL[gt]:GCOL[gt] + 1])
                            act[gt] = a
                    t1 = wpool.tile([2 * H, NB], BF16, tag=f"t1{g}", name=f"t1{g}")
                    nc.vector.tensor_tensor(t1[lo:hi, :], act["f"][lo:hi, :],
                                            c_st[g][lo:hi, :], op=OP.mult)
                    t2 = wpool.tile([2 * H, NB], BF16, tag=f"t2{g}", name=f"t2{g}")
                    nc.vector.tensor_tensor(t2[lo:hi, :], act["i"][lo:hi, :],
                                            act["g"][lo:hi, :], op=OP.mult)
                    nc.gpsimd.tensor_tensor(c_st[g][lo:hi, :], t1[lo:hi, :],
                                            t2[lo:hi, :], op=OP.add)
                    tC = wpool.tile([2 * H, NB], BF16, tag=f"tC{g}", name=f"tC{g}")
                    nc.scalar.activation(tC[lo:hi, :], c_st[g][lo:hi, :], AF.Tanh)
                    if s < T:
                        # h0 directly into layer0's next rhs (parallel with the
                        # main h-update below, instead of a dependent copy)
                        nc.vector.tensor_tensor(rhsA[g][nxt][0:H, :],
                                                act["o"][0:H, :], tC[0:H, :],
                                                op=OP.mult)
                        nc.vector.tensor_tensor(rhsB[g][nxt][lo:hi, :],
                                                act["o"][lo:hi, :], tC[lo:hi, :],
                                                op=OP.mult)
                        if s + 1 < T:
                            nc.sync.dma_start(
                                rhsA[g][nxt][H:H + 1, :],
                                xT[s + 1:s + 2, g * NB:(g + 1) * NB])
                    else:
                        nc.vector.tensor_tensor(h1f[g][:], act["o"][H:2 * H, :],
                                                tC[H:2 * H, :], op=OP.mult)

            for s in range(T + 1):
                step(s)

            # final linear: out[b, :] = h1f.T @ wlin + blin
            for g in range(NG):
                for blk in range(NB // 128):
                    psl = ppool.tile([128, H], F32, tag="ps_f0", name="psl")
                    nc.tensor.matmul(psl[:], h1f[g][:, blk * 128:(blk + 1) * 128],
                                     wlin_s[:], start=True, stop=True)
                    ob = wpool.tile([128, H], F32, tag="ob", name="ob")
                    import concourse.mybir as _mb
                    nc.vector.scalar_tensor_tensor(ob[:], psl[:], 1.0,
                                                   blin_s[0:128, :],
                                                   op0=OP.mult, op1=OP.add)
                    row0 = g * NB + blk * 128
                    nc.sync.dma_start(out[row0:row0 + 128, :], ob[:])

    _split_excess_waits(nc, limit=1)
    return nc


def _prep_inputs(inputs):
    bf = ml_dtypes.bfloat16
    recipe = np.ascontiguousarray(np.asarray(inputs["recipe"], np.float32).reshape(B, T))
    w0 = np.concatenate([np.asarray(inputs["W_hh0"]).T,
                         np.asarray(inputs["W_ih0"]).T], axis=0).astype(bf)
    w1 = np.concatenate([np.asarray(inputs["W_ih1"]).T,
                         np.asarray(inputs["W_hh1"]).T], axis=0).astype(bf)
    b0 = (np.asarray(inputs["b_ih0"]) + np.asarray(inputs["b_hh0"])).astype(np.float32)
    b1 = (np.asarray(inputs["b_ih1"]) + np.asarray(inputs["b_hh1"])).astype(np.float32)
    bias = np.zeros((2 * H, 4), np.float32)
    for gi in range(4):
        bias[0:H, gi] = b0[gi * H:(gi + 1) * H]
        bias[H:2 * H, gi] = b1[gi * H:(gi + 1) * H]
    wlin = np.ascontiguousarray(np.asarray(inputs["W_lin"]).T).astype(bf)
    blin = np.tile(np.asarray(inputs["b_lin"], np.float32), (2 * H, 1))
    in_maps = []
    for i in range(N_CORES):
        shard = recipe[i * B_LOC:(i + 1) * B_LOC]
        xTs = np.ascontiguousarray(shard.T).astype(bf)
        in_maps.append({"xT": xTs, "w0": w0, "w1": w1, "bias": bias,
                        "wlin": wlin, "blin": blin})
    return in_maps


_PROGRAM = []


def _run(inputs, trace=False):
    from concourse.bass_utils import run_bass_kernel_spmd
    if not _PROGRAM:
        _PROGRAM.append(_build_program())
    nc = _PROGRAM[0]
    in_maps = _prep_inputs(inputs)
    last_err = None
    for attempt in range(3):
        try:
            res = run_bass_kernel_spmd(nc, in_maps,
                                       core_ids=list(range(N_CORES)), trace=trace)
            outs = [np.asarray(res.results[i]["out"]) for i in range(N_CORES)]
            return np.concatenate(outs, axis=0), res
        except Exception as e:  # transient first-exec device faults: retry
            last_err = e
    raise last_err


def kernel(**inputs):
    full, _ = _run(inputs, trace=False)
    return full.astype(np.float32)



# revision 12
# speedup vs baseline: 1.5215x; 1.0646x over previous
"""Trainium2 Bass kernel for nn_AIGStateEncoder (2-layer LSTM + linear head).

Data-parallel over batch: B=4096 rows split across 8 NeuronCores (512 each).
Per core the two LSTM layers are fused into one recurrence ("combined step"
s runs layer0 at t=s and layer1 at t=s-1), with the state kept transposed
(hidden units on SBUF partitions, batch on the free dimension) and the two
layers stacked on the 128 partitions: [layer0 (0:64); layer1 (64:128)].

Per combined step and batch-group (2 independent groups of 256 batch rows
hide each other's serial-dependency chain):
  - 8 matmuls (4 gates x 2 layers; K=65 for layer0 ([x_t; h0]), K=128 for
    layer1 ([h0; h1]); N=256) into per-gate PSUM banks,
  - 4 sigmoid/tanh ScalarE activations with per-partition bias (both layers
    in one [128, 256] instruction each),
  - cell/hidden updates on VectorE in bf16 (2x mode),
  - tanh(c) on ScalarE.
The per-step x row is DMA'd from DRAM into partition 64 of the layer0
moving operand; h0 is copied into it by VectorE.

Everything computes in bf16 (fp32 PSUM accumulation); measured end-to-end
relative error vs the fp32 reference is ~3e-3.
"""
import sys

if '/opt/trn_rl_repo' not in sys.path:
    sys.path.insert(0, '/opt/trn_rl_repo')

import numpy as np
import ml_dtypes

B, T, H = 4096, 256, 64
N_CORES = 8
B_LOC = B // N_CORES  # 512

GATE_ORDER = ("f", "g", "i", "o")   # emission order (c-critical gates first)
GCOL = {"i": 0, "f": 1, "g": 2, "o": 3}  # PyTorch gate order i,f,g,o


def _split_excess_waits(nc, limit=1):
    """The walrus build in this container accepts at most one sync wait per
    instruction.  Hoist excess waits onto NoOps inserted just before the
    instruction on the same engine (same-engine program order preserves the
    synchronization semantics)."""
    import concourse.mybir as mybir
    ctr = 0
    for f in nc.m.functions:
        for bb in f.blocks:
            il = bb.instructions
            i = 0
            while i < len(il):
                ins = il[i]
                si = ins.sync_info
                if si is not None and si.on_wait and len(si.on_wait) > limit:
                    waits = list(si.on_wait)
                    excess, keep = waits[:-limit], waits[-limit:]
                    while excess:
                        chunk, excess = excess[:limit], excess[limit:]
                        nop = mybir.InstNoOp(name=f"waitsplit_{ctr}", ins=[], outs=[])
                        ctr += 1
                        nop.engine = ins.engine
                        nop.sync_info = mybir.SyncInfo(on_wait=chunk, on_update=[])
                        il.insert(i, nop)
                        i += 1
                    ins.sync_info = mybir.SyncInfo(on_wait=keep,
                                                   on_update=list(si.on_update))
                i += 1


def _build_program(n_groups=2):
    import concourse.bass as bass
    import concourse.mybir as mybir
    from concourse.tile import TileContext

    BF16 = mybir.dt.bfloat16
    F32 = mybir.dt.float32
    AF = mybir.ActivationFunctionType
    OP = mybir.AluOpType
    GFUNC = {"i": AF.Sigmoid, "f": AF.Sigmoid, "g": AF.Tanh, "o": AF.Sigmoid}

    NG = n_groups
    NB = B_LOC // NG

    nc = bass.Bass()
    xT = nc.declare_dram_parameter("xT", [T, B_LOC], BF16, isOutput=False)
    w0 = nc.declare_dram_parameter("w0", [1 + H, 4 * H], BF16, isOutput=False)
    w1 = nc.declare_dram_parameter("w1", [2 * H, 4 * H], BF16, isOutput=False)
    bias = nc.declare_dram_parameter("bias", [2 * H, 4], F32, isOutput=False)
    wlin = nc.declare_dram_parameter("wlin", [H, H], BF16, isOutput=False)
    blin = nc.declare_dram_parameter("blin", [2 * H, H], F32, isOutput=False)
    out = nc.declare_dram_parameter("out", [B_LOC, H], F32, isOutput=True)

    with TileContext(nc) as tc:
        with (
            tc.tile_pool(name="const", bufs=1) as cpool,
            tc.tile_pool(name="state", bufs=1) as spool,
            tc.tile_pool(name="work", bufs=3) as wpool,
            tc.tile_pool(name="psum", bufs=1, space="PSUM") as ppool,
        ):
            w0_s = cpool.tile([1 + H, 4 * H], BF16, tag="w0", name="w0")
            nc.sync.dma_start(w0_s[:], w0[:])
            w1_s = cpool.tile([2 * H, 4 * H], BF16, tag="w1", name="w1")
            nc.sync.dma_start(w1_s[:], w1[:])
            bias_s = cpool.tile([2 * H, 4], F32, tag="bias", name="bias")
            nc.sync.dma_start(bias_s[:], bias[:])
            wlin_s = cpool.tile([H, H], BF16, tag="wlin", name="wlin")
            nc.sync.dma_start(wlin_s[:], wlin[:])
            blin_s = cpool.tile([2 * H, H], F32, tag="blin", name="blin")
            nc.sync.dma_start(blin_s[:], blin[:])

            rhsA = []  # [65, NB] : [h0 (0:64); x_t (64)]
            rhsB = []  # [128, NB]: [h0; h1]
            c_st = []  # [128, NB]: [c0; c1]
            h1f = []   # [64, NB] : final h1
            for g in range(NG):
                rhsA.append([spool.tile([1 + H, NB], BF16, tag=f"rhsA{g}_{p}",
                                        name=f"rhsA{g}_{p}") for p in range(2)])
                rhsB.append([spool.tile([2 * H, NB], BF16, tag=f"rhsB{g}_{p}",
                                        name=f"rhsB{g}_{p}") for p in range(2)])
                c_st.append(spool.tile([2 * H, NB], BF16, tag=f"c{g}", name=f"cst{g}"))
                h1f.append(spool.tile([H, NB], BF16, tag=f"h1f{g}", name=f"h1f{g}"))
                for p in range(2):
                    nc.gpsimd.memset(rhsA[g][p][:], 0.0)
                    nc.gpsimd.memset(rhsB[g][p][:], 0.0)
                nc.gpsimd.memset(c_st[g][:], 0.0)
                nc.sync.dma_start(rhsA[g][0][H:H + 1, :],
                                  xT[0:1, g * NB:(g + 1) * NB])

            def step(s):
                cur, nxt = s % 2, (s + 1) % 2
                l0 = s < T
                l1 = s >= 1
                lo = 0 if l0 else H
                hi = 2 * H if l1 else H
                for g in range(NG):
                    ps = {}
                    for gt in GATE_ORDER:
                        ps[gt] = ppool.tile([2 * H, NB], F32, tag=f"ps_{gt}{g}",
                                            name=f"ps_{gt}{g}")
                        col = GCOL[gt] * H
                        if l0:
                            nc.tensor.matmul(ps[gt][0:H, :], w0_s[:, col:col + H],
                                             rhsA[g][cur][:], start=True, stop=True)
                        if l1:
                            nc.tensor.matmul(ps[gt][H:2 * H, :], w1_s[:, col:col + H],
                                             rhsB[g][cur][:], start=True, stop=True)
                    # f,o: in-place PSUM activations (ScalarE is closer to
                    # PSUM; 172-cycle access vs 222 for SBUF) -- their DVE
                    # consumers pair them with an SBUF operand.  i,g keep
                    # SBUF outputs (t2 = i*g may read only one PSUM operand).
                    act = {}
                    for gt in GATE_ORDER:
                        if gt in ("f", "o"):
                            nc.scalar.activation(ps[gt][lo:hi, :], ps[gt][lo:hi, :],
                                                 GFUNC[gt],
                                                 bias=bias_s[lo:hi, GCOL[gt]:GCOL[gt] + 1])
                            act[gt] = ps[gt]
                        else:
                            a = wpool.tile([2 * H, NB], BF16, tag=f"a_{gt}{g}",
                                           name=f"a_{gt}{g}")
                            nc.scalar.activation(a[lo:hi, :], ps[gt][lo:hi, :],
                                                 GFUNC[gt],
                                                 bias=bias_s[lo:hi, GCOL[gt]:GCOL[gt] + 1])
                            act[gt] = a
                    t1 = wpool.tile([2 * H, NB], BF16, tag=f"t1{g}", name=f"t1{g}")
                    nc.vector.tensor_tensor(t1[lo:hi, :], act["f"][lo:hi, :],
                                            c_st[g][lo:hi, :], op=OP.mult)
                    t2 = wpool.tile([2 * H, NB], BF16, tag=f"t2{g}", name=f"t2{g}")
                    nc.vector.tensor_tensor(t2[lo:hi, :], act["i"][lo:hi, :],
                                            act["g"][lo:hi, :], op=OP.mult)
                    nc.vector.tensor_tensor(c_st[g][lo:hi, :], t1[lo:hi, :],
                                            t2[lo:hi, :], op=OP.add)
                    tC = wpool.tile([2 * H, NB], BF16, tag=f"tC{g}", name=f"tC{g}")
                    nc.scalar.activation(tC[lo:hi, :], c_st[g][lo:hi, :], AF.Tanh)
                    if s < T:
                        # h0 directly into layer0's next rhs (parallel with the
                        # main h-update below, instead of a dependent copy)
                        nc.vector.tensor_tensor(rhsA[g][nxt][0:H, :],
                                                act["o"][0:H, :], tC[0:H, :],
                                                op=OP.mult)
                        nc.vector.tensor_tensor(rhsB[g][nxt][lo:hi, :],
                                                act["o"][lo:hi, :], tC[lo:hi, :],
                                                op=OP.mult)
                        if s + 1 < T:
                            nc.sync.dma_start(
                                rhsA[g][nxt][H:H + 1, :],
                                xT[s + 1:s + 2, g * NB:(g + 1) * NB])
                    else:
                        nc.vector.tensor_tensor(h1f[g][:], act["o"][H:2 * H, :],
                                                tC[H:2 * H, :], op=OP.mult)

            for s in range(T + 1):
                step(s)

            # final linear: out[b, :] = h1f.T @ wlin + blin
            for g in range(NG):
                for blk in range(NB // 128):
                    psl = ppool.tile([128, H], F32, tag="ps_f0", name="psl")
                    nc.tensor.matmul(psl[:], h1f[g][:, blk * 128:(blk + 1) * 128],
                                     wlin_s[:], start=True, stop=True)
                    ob = wpool.tile([128, H], F32, tag="ob", name="ob")
                    import concourse.mybir as _mb
                    nc.vector.scalar_tensor_tensor(ob[:], psl[:], 1.0,
                                                   blin_s[0:128, :],
                                                   op0=OP.mult, op1=OP.add)
                    row0 = g * NB + blk * 128
                    nc.sync.dma_start(out[row0:row0 + 128, :], ob[:])

    _split_excess_waits(nc, limit=1)
    return nc


def _prep_inputs(inputs):
    bf = ml_dtypes.bfloat16
    recipe = np.ascontiguousarray(np.asarray(inputs["recipe"], np.float32).reshape(B, T))
    w0 = np.concatenate([np.asarray(inputs["W_hh0"]).T,
                         np.asarray(inputs["W_ih0"]).T], axis=0).astype(bf)
    w1 = np.concatenate([np.asarray(inputs["W_ih1"]).T,
                         np.asarray(inputs["W_hh1"]).T], axis=0).astype(bf)
    b0 = (np.asarray(inputs["b_ih0"]) + np.asarray(inputs["b_hh0"])).astype(np.float32)
    b1 = (np.asarray(inputs["b_ih1"]) + np.asarray(inputs["b_hh1"])).astype(np.float32)
    bias = np.zeros((2 * H, 4), np.float32)
    for gi in range(4):
        bias[0:H, gi] = b0[gi * H:(gi + 1) * H]
        bias[H:2 * H, gi] = b1[gi * H:(gi + 1) * H]
    wlin = np.ascontiguousarray(np.asarray(inputs["W_lin"]).T).astype(bf)
    blin = np.tile(np.asarray(inputs["b_lin"], np.float32), (2 * H, 1))
    in_maps = []
    for i in range(N_CORES):
        shard = recipe[i * B_LOC:(i + 1) * B_LOC]
        xTs = np.ascontiguousarray(shard.T).astype(bf)
        in_maps.append({"xT": xTs, "w0": w0, "w1": w1, "bias": bias,
                        "wlin": wlin, "blin": blin})
    return in_maps


_PROGRAM = []


def _run(inputs, trace=False):
    from concourse.bass_utils import run_bass_kernel_spmd
    if not _PROGRAM:
        _PROGRAM.append(_build_program())
    nc = _PROGRAM[0]
    in_maps = _prep_inputs(inputs)
    last_err = None
    for attempt in range(3):
        try:
            res = run_bass_kernel_spmd(nc, in_maps,
                                       core_ids=list(range(N_CORES)), trace=trace)
            outs = [np.asarray(res.results[i]["out"]) for i in range(N_CORES)]
            return np.concatenate(outs, axis=0), res
        except Exception as e:  # transient first-exec device faults: retry
            last_err = e
    raise last_err


def kernel(**inputs):
    full, _ = _run(inputs, trace=False)
    return full.astype(np.float32)

